# revision 1
# baseline (speedup 1.0000x reference)
"""Trainium2 Bass kernel for L4Q quantized linear (LoRA + group fake-quant + GEMM).

Computation (per reference):
    w   = w0 + lora_b @ lora_a                      # [4096, 4096]
    w_q = round(clip(w/s, -8, 7)) * s               # group-wise (groups of 128 along in)
    y   = x @ w_q.T + bias                          # x: [4, 2048, 4096]

Sharding: column-parallel over out_features across 8 cores (512 outs/core).
x is replicated (pre-transposed + fp16-cast on host); each core computes
y[:, :, c*512:(c+1)*512] and the host concatenates.

Numeric strategy:
  - dequant runs in exact fp32 on-device (PE fp32 matmul for the K=16 LoRA
    delta, DVE IEEE fp32 elementwise, magic-number round-half-even,
    NR-refined reciprocal) so quantization decisions match the fp32 reference
    to ~1 ulp.
  - the big GEMM runs in fp16 (11-bit mantissa) with fp32 PSUM accumulation:
    ~2e-4 scale-relative absmax error, at full PE rate (1 elem/cycle).
"""
import numpy as np

import concourse.bass as bass
import concourse.bacc as bacc
import concourse.mybir as mybir
from concourse.tile import TileContext
from concourse.bass_utils import run_bass_kernel_spmd
from concourse.alu_op_type import AluOpType

F32 = mybir.dt.float32
F16 = mybir.dt.float16
MAGIC = 12582912.0  # 1.5 * 2**23: forces round-to-nearest-even at integer granularity

N_CORES = 8
IN_F = 4096
OUT_F = 4096
RANK = 16
B, S = 4, 2048
M_TOK = B * S            # 8192 tokens
OUT_SH = OUT_F // N_CORES  # 512 out features per core
GROUP = 128
N_GROUPS = IN_F // GROUP   # 32 k-tiles
TOK_CHUNK = 512            # tokens per x-slab DMA
N_CHUNKS = M_TOK // TOK_CHUNK  # 16
Q_N, Q_P = -8.0, 7.0

_CACHE = {}


def _build():
    nc = bacc.Bacc(None, target_bir_lowering=False)
    xT_d = nc.dram_tensor("xT16", [IN_F, M_TOK], F16, kind="ExternalInput")
    w0T_d = nc.dram_tensor("w0T", [IN_F, OUT_SH], F32, kind="ExternalInput")
    la_d = nc.dram_tensor("lora_a", [RANK, IN_F], F32, kind="ExternalInput")
    lbT_d = nc.dram_tensor("lora_bT", [RANK, OUT_SH], F32, kind="ExternalInput")
    qsT_d = nc.dram_tensor("qscT", [N_GROUPS, OUT_SH], F32, kind="ExternalInput")
    bias_d = nc.dram_tensor("bias", [1, OUT_SH], F32, kind="ExternalInput")
    y_d = nc.dram_tensor("y", [M_TOK, OUT_SH], F32, kind="ExternalOutput")

    with TileContext(nc) as tc:
        with (
            tc.tile_pool(name="persist", bufs=1) as persist,
            tc.tile_pool(name="w0", bufs=2) as w0pool,
            tc.tile_pool(name="deq", bufs=3) as deq,
            tc.tile_pool(name="xslab", bufs=2) as xpool,
            tc.tile_pool(name="ystage", bufs=2) as ypool,
            tc.tile_pool(name="pdeq", bufs=2, space="PSUM") as pdeq,
            tc.tile_pool(name="pbc", bufs=2, space="PSUM") as pbc,
            tc.tile_pool(name="pmm", bufs=2, space="PSUM") as pmm,
            tc.tile_pool(name="dram", bufs=1, space="DRAM") as dram,
        ):
            # ---------- setup ----------
            ones_sb = persist.tile([1, 128], F32)
            nc.vector.memset(ones_sb[:], 1.0)

            la_sb = persist.tile([RANK, IN_F], F32)
            nc.sync.dma_start(la_sb[:], la_d[:, :])
            lbT_sb = persist.tile([RANK, OUT_SH], F32)
            nc.sync.dma_start(lbT_sb[:], lbT_d[:, :])

            # scales: s [32, 512]; r = 1/s via reciprocal + 2 NR (0-ulp exact
            # per HW probe)
            sT32 = persist.tile([N_GROUPS, OUT_SH], F32)
            nc.sync.dma_start(sT32[:], qsT_d[:, :])
            r32 = persist.tile([N_GROUPS, OUT_SH], F32)
            nc.vector.reciprocal(r32[:], sT32[:])
            t32 = persist.tile([N_GROUPS, OUT_SH], F32)
            for _ in range(2):
                nc.vector.tensor_tensor(t32[:], sT32[:], r32[:], AluOpType.mult)
                nc.vector.tensor_scalar(t32[:], t32[:], -1.0, 2.0,
                                        AluOpType.mult, AluOpType.add)
                nc.vector.tensor_tensor(r32[:], r32[:], t32[:], AluOpType.mult)
            r_dram = dram.tile([N_GROUPS, OUT_SH], F32)
            nc.sync.dma_start(r_dram[:], r32[:])

            # bias broadcast tile [128, OUT_SH]
            biasT_sb = persist.tile([1, OUT_SH], F32)
            nc.sync.dma_start(biasT_sb[:], bias_d[:, :])
            bias_ps = pdeq.tile([128, OUT_SH], F32, tag="dps")
            nc.tensor.matmul(bias_ps[:], ones_sb[:], biasT_sb[:],
                             start=True, stop=True)
            bias_bc = persist.tile([128, OUT_SH], F32)
            nc.vector.tensor_copy(bias_bc[:], bias_ps[:])

            # ---------- phase 1: dequantize wT into fp16 ----------
            # persistent fp16 weight slab [128, 32, 512]
            wt16 = persist.tile([128, N_GROUPS, OUT_SH], F16)
            W0_BATCH = 4  # k-tiles per w0T DMA (1 MiB transfers)
            for kb in range(N_GROUPS // W0_BATCH):
                w0_sb = w0pool.tile([128, W0_BATCH, OUT_SH], F32, tag="w0")
                nc.sync.dma_start(
                    w0_sb[:],
                    w0T_d.rearrange("(kb p) o -> p kb o", p=128)[
                        :, kb * W0_BATCH:(kb + 1) * W0_BATCH, :])
                # stage scale/recip rows onto partition 0 for the broadcast MMs
                srow = deq.tile([1, W0_BATCH, OUT_SH], F32, tag="srow", bufs=2)
                nc.sync.dma_start(
                    srow[:], qsT_d[kb * W0_BATCH:(kb + 1) * W0_BATCH, :][None])
                rrow = deq.tile([1, W0_BATCH, OUT_SH], F32, tag="rrow", bufs=2)
                nc.sync.dma_start(
                    rrow[:], r_dram[kb * W0_BATCH:(kb + 1) * W0_BATCH, :][None])
                for ki in range(W0_BATCH):
                    k = kb * W0_BATCH + ki
                    # lora delta^T tile via fp32 PE matmul (K=16)
                    d_ps = pdeq.tile([128, OUT_SH], F32, tag="dps")
                    nc.tensor.matmul(d_ps[:], la_sb[:, k * 128:(k + 1) * 128],
                                     lbT_sb[:], start=True, stop=True)
                    # broadcast scale row k and reciprocal row k to 128 partitions
                    s_ps = pbc.tile([128, OUT_SH], F32, tag="sps")
                    nc.tensor.matmul(s_ps[:], ones_sb[:], srow[0:1, ki, :],
                                     start=True, stop=True)
                    r_ps = pbc.tile([128, OUT_SH], F32, tag="rps")
                    nc.tensor.matmul(r_ps[:], ones_sb[:], rrow[0:1, ki, :],
                                     start=True, stop=True)
                    # w = w0 + delta  (exact fp32)
                    v = deq.tile([128, OUT_SH], F32, tag="v")
                    nc.vector.tensor_tensor(v[:], d_ps[:], w0_sb[:, ki, :],
                                            AluOpType.add)
                    # v = w * (1/s)
                    nc.vector.tensor_tensor(v[:], v[:], r_ps[:], AluOpType.mult)
                    # clip to [-8, 7]
                    nc.vector.tensor_scalar(v[:], v[:], Q_N, Q_P,
                                            AluOpType.max, AluOpType.min)
                    # round half-to-even
                    nc.vector.tensor_scalar(v[:], v[:], MAGIC, MAGIC,
                                            AluOpType.add, AluOpType.subtract)
                    # w_q = q * s, cast to fp16
                    nc.vector.tensor_tensor(wt16[:, k, :], v[:], s_ps[:],
                                            AluOpType.mult)

            # ---------- phase 2: GEMM ----------
            for c in range(N_CHUNKS):
                xs = xpool.tile([128, N_GROUPS, TOK_CHUNK], F16, tag="xs")
                nc.sync.dma_start(
                    xs[:],
                    xT_d.rearrange("(kb p) m -> p kb m", p=128)[
                        :, :, c * TOK_CHUNK:(c + 1) * TOK_CHUNK])
                y_sb = ypool.tile([128, TOK_CHUNK // 128, OUT_SH], F32, tag="y")
                for t in range(TOK_CHUNK // 128):
                    y_ps = pmm.tile([128, OUT_SH], F32, tag="yps")
                    for k in range(N_GROUPS):
                        nc.tensor.matmul(y_ps[:],
                                         xs[:, k, t * 128:(t + 1) * 128],
                                         wt16[:, k, :],
                                         start=(k == 0), stop=(k == N_GROUPS - 1))
                    # bias add + psum drain in one DVE pass
                    nc.vector.tensor_tensor(y_sb[:, t, :], y_ps[:], bias_bc[:],
                                            AluOpType.add)
                nc.sync.dma_start(
                    y_d.rearrange("(c t p) o -> c p t o", p=128,
                                  t=TOK_CHUNK // 128)[c],
                    y_sb[:])
    nc.compile()
    return nc


def _make_in_maps(x, w0, lora_a, lora_b, q_scale, bias):
    # host-side layout marshalling (no arithmetic beyond the fp16 cast of x,
    # which is the kernel's chosen input precision for the tensor engine)
    x = np.ascontiguousarray(np.asarray(x, dtype=np.float32))
    xT16 = np.ascontiguousarray(x.reshape(M_TOK, IN_F).T).astype(np.float16)
    w0T = np.ascontiguousarray(np.asarray(w0, dtype=np.float32).T)
    lbT = np.ascontiguousarray(np.asarray(lora_b, dtype=np.float32).T)
    qs2 = np.asarray(q_scale, dtype=np.float32).reshape(OUT_F, N_GROUPS)
    bias = np.asarray(bias, dtype=np.float32)
    lora_a = np.ascontiguousarray(np.asarray(lora_a, dtype=np.float32))
    in_maps = []
    for c in range(N_CORES):
        sl = slice(c * OUT_SH, (c + 1) * OUT_SH)
        in_maps.append({
            "xT16": xT16,
            "w0T": np.ascontiguousarray(w0T[:, sl]),
            "lora_a": lora_a,
            "lora_bT": np.ascontiguousarray(lbT[:, sl]),
            "qscT": np.ascontiguousarray(qs2[sl].T),
            "bias": np.ascontiguousarray(bias[sl]).reshape(1, OUT_SH),
        })
    return in_maps


def kernel(x, w0, lora_a, lora_b, q_scale, bias):
    if "nc" not in _CACHE:
        _CACHE["nc"] = _build()
    in_maps = _make_in_maps(x, w0, lora_a, lora_b, q_scale, bias)
    res = run_bass_kernel_spmd(_CACHE["nc"], in_maps,
                               core_ids=list(range(N_CORES)))
    y = np.concatenate([res.results[c]["y"] for c in range(N_CORES)], axis=1)
    return y.reshape(B, S, OUT_F)


def timed_run(inputs):
    """Profiled run for test.py: returns max-core HW exec time in ns."""
    if "nc" not in _CACHE:
        _CACHE["nc"] = _build()
    in_maps = _make_in_maps(**inputs)
    res = run_bass_kernel_spmd(
        _CACHE["nc"], in_maps, core_ids=list(range(N_CORES)),
        trace=True, trace_cores=list(range(N_CORES)))
    print("per-core exec ns:", res.mean_exec_time_ns, "max core:",
          res.max_exec_time_core_id)
    if res.instructions_and_trace:
        insts, path = res.instructions_and_trace
        print("trace path:", path)
        if insts:
            t0 = min(i.timestamp for i in insts)
            t1 = max(i.end_timestamp for i in insts)
            span = t1 - t0
            from collections import defaultdict
            busy = defaultdict(int)
            cnt = defaultdict(int)
            for i in insts:
                busy[i.engine] += i.duration
                cnt[i.engine] += 1
            print(f"span: {span} ns")
            for e in sorted(busy, key=lambda e: -busy[e]):
                print(f"  {e:>10}: busy {busy[e]:>9} ns ({100.0*busy[e]/span:5.1f}%)"
                      f"  n={cnt[e]}")
            byop = defaultdict(int)
            for i in insts:
                byop[(i.engine, i.op_name)] += i.duration
            top = sorted(byop.items(), key=lambda kv: -kv[1])[:10]
            for (e, op), d in top:
                print(f"    {e}/{op}: {d} ns")
    return res.exec_time_ns



# revision 6
# speedup vs baseline: 1.0274x; 1.0274x over previous
"""Trainium2 Bass kernel for L4Q quantized linear (LoRA + group fake-quant + GEMM).

Computation (per reference):
    w   = w0 + lora_b @ lora_a                      # [4096, 4096]
    w_q = round(clip(w/s, -8, 7)) * s               # group-wise (groups of 128 along in)
    y   = x @ w_q.T + bias                          # x: [4, 2048, 4096]

Sharding: column-parallel over out_features across 8 cores (512 outs/core).
x is replicated (pre-transposed + fp16-cast on host); each core computes
y[:, :, c*512:(c+1)*512] and the host concatenates.

Numeric strategy:
  - dequant runs in exact fp32 on-device (PE fp32 matmul for the K=16 LoRA
    delta, DVE IEEE fp32 elementwise, magic-number round-half-even,
    NR-refined reciprocal) so quantization decisions match the fp32 reference
    to ~1 ulp.
  - the big GEMM runs in fp16 (11-bit mantissa) with fp32 PSUM accumulation.

Pipelining (v2): K is split into 4 quarters of 8 k-tiles. Quarter 0 is
dequantized up front; quarter q+1's dequant (DVE-heavy) is interleaved under
quarter q's GEMM matmuls so the PE never idles long enough for the HAM clock
throttle to drop it out of the 2.4 GHz p-state (the v1 kernel spent ~150us
running dequant matmuls at the 0.65 GHz cold p-state, fully serialized before
the GEMM). Partial sums accumulate chunk-wise in an fp16 SBUF accumulator.
"""
import numpy as np

import concourse.bass as bass
import concourse.bacc as bacc
import concourse.mybir as mybir
from concourse.tile import TileContext
from concourse.bass_utils import run_bass_kernel_spmd
from concourse.alu_op_type import AluOpType

F32 = mybir.dt.float32
F16 = mybir.dt.float16
MAGIC = 12582912.0  # 1.5 * 2**23: forces round-to-nearest-even at integer granularity

N_CORES = 8
IN_F = 4096
OUT_F = 4096
RANK = 16
B, S = 4, 2048
M_TOK = B * S            # 8192 tokens
OUT_SH = OUT_F // N_CORES  # 512 out features per core
GROUP = 128
N_KT = IN_F // GROUP       # 32 k-tiles (1 quant group per k-tile)
NQ = 4                     # K quarters
QK = N_KT // NQ            # 8 k-tiles per quarter
TOK_CHUNK = 512            # tokens per x-slab DMA
N_CHUNKS = M_TOK // TOK_CHUNK  # 16
N_TT = M_TOK // 128        # 64 token tiles
Q_N, Q_P = -8.0, 7.0

_CACHE = {}


def _build():
    nc = bacc.Bacc(None, target_bir_lowering=False)
    xT_d = nc.dram_tensor("xT16", [IN_F, M_TOK], F16, kind="ExternalInput")
    w0T_d = nc.dram_tensor("w0T", [IN_F, OUT_SH], F32, kind="ExternalInput")
    la_d = nc.dram_tensor("lora_a", [RANK, IN_F], F32, kind="ExternalInput")
    lbT_d = nc.dram_tensor("lora_bT", [RANK, OUT_SH], F32, kind="ExternalInput")
    qsT_d = nc.dram_tensor("qscT", [N_KT, OUT_SH], F32, kind="ExternalInput")
    bias_d = nc.dram_tensor("bias", [1, OUT_SH], F32, kind="ExternalInput")
    y_d = nc.dram_tensor("y", [M_TOK, OUT_SH], F32, kind="ExternalOutput")

    with TileContext(nc) as tc:
        with (
            tc.tile_pool(name="persist", bufs=1) as persist,
            tc.tile_pool(name="wtq", bufs=2) as wtqpool,
            tc.tile_pool(name="w0", bufs=2) as w0pool,
            tc.tile_pool(name="deq", bufs=2) as deq,
            tc.tile_pool(name="xslab", bufs=2) as xpool,
            tc.tile_pool(name="yout", bufs=2) as ypool,
            tc.tile_pool(name="pdeq", bufs=2, space="PSUM") as pdeq,
            tc.tile_pool(name="pbc", bufs=2, space="PSUM") as pbc,
            tc.tile_pool(name="pmm", bufs=2, space="PSUM") as pmm,
            tc.tile_pool(name="dram", bufs=1, space="DRAM") as dram,
        ):
            # ---------- setup ----------
            ones_sb = persist.tile([1, 128], F32)
            nc.vector.memset(ones_sb[:], 1.0)

            la_sb = persist.tile([RANK, IN_F], F32)
            nc.sync.dma_start(la_sb[:], la_d[:, :])
            lbT_sb = persist.tile([RANK, OUT_SH], F32)
            nc.sync.dma_start(lbT_sb[:], lbT_d[:, :])

            # scales: s [32, 512]; r = 1/s via reciprocal + 2 NR (0-ulp exact
            # per HW probe)
            sT32 = persist.tile([N_KT, OUT_SH], F32)
            nc.sync.dma_start(sT32[:], qsT_d[:, :])
            r32 = persist.tile([N_KT, OUT_SH], F32)
            nc.vector.reciprocal(r32[:], sT32[:])
            t32 = persist.tile([N_KT, OUT_SH], F32)
            for _ in range(2):
                nc.vector.tensor_tensor(t32[:], sT32[:], r32[:], AluOpType.mult)
                nc.vector.tensor_scalar(t32[:], t32[:], -1.0, 2.0,
                                        AluOpType.mult, AluOpType.add)
                nc.vector.tensor_tensor(r32[:], r32[:], t32[:], AluOpType.mult)
            r_dram = dram.tile([N_KT, OUT_SH], F32)
            nc.sync.dma_start(r_dram[:], r32[:])

            # bias broadcast tile [128, OUT_SH] fp32
            biasT_sb = persist.tile([1, OUT_SH], F32)
            nc.sync.dma_start(biasT_sb[:], bias_d[:, :])
            bias_ps = pdeq.tile([128, OUT_SH], F32, tag="dps")
            nc.tensor.matmul(bias_ps[:], ones_sb[:], biasT_sb[:],
                             start=True, stop=True)
            bias_bc = persist.tile([128, OUT_SH], F32)
            nc.vector.tensor_copy(bias_bc[:], bias_ps[:])

            # fp16 partial-sum accumulator, one [128, OUT_SH] slab per token tile
            y16 = persist.tile([128, N_TT, OUT_SH], F16)

            w0T_r = w0T_d.rearrange("(kt p) o -> p kt o", p=128)
            xT_r = xT_d.rearrange("(kt p) m -> p kt m", p=128)
            y_r = y_d.rearrange("(n p) o -> n p o", p=128)

            wtq_tiles = {}

            def deq_dma(q, pair):
                """DMA w0/scale/recip rows for k-tiles (q*QK + 2*pair), +1."""
                k0 = q * QK + 2 * pair
                w0_sb = w0pool.tile([128, 2, OUT_SH], F32, tag="w0")
                nc.sync.dma_start(w0_sb[:], w0T_r[:, k0:k0 + 2, :])
                srow = deq.tile([1, 2, OUT_SH], F32, tag="srow")
                nc.sync.dma_start(srow[:], qsT_d[k0:k0 + 2, :][None])
                rrow = deq.tile([1, 2, OUT_SH], F32, tag="rrow")
                nc.sync.dma_start(rrow[:], r_dram[k0:k0 + 2, :][None])
                return w0_sb, srow, rrow

            def deq_k(q, pair, ki, bufs):
                """Dequantize k-tile q*QK + 2*pair + ki into wtq_tiles[q]."""
                w0_sb, srow, rrow = bufs
                wtq = wtq_tiles[q]
                k = q * QK + 2 * pair + ki
                kl = 2 * pair + ki
                if True:
                    # lora delta^T tile via fp32 PE matmul (K=16)
                    d_ps = pdeq.tile([128, OUT_SH], F32, tag="dps")
                    nc.tensor.matmul(d_ps[:], la_sb[:, k * 128:(k + 1) * 128],
                                     lbT_sb[:], start=True, stop=True)
                    # broadcast scale/recip rows k to 128 partitions
                    s_ps = pbc.tile([128, OUT_SH], F32, tag="sps")
                    nc.tensor.matmul(s_ps[:], ones_sb[:], srow[0:1, ki, :],
                                     start=True, stop=True)
                    r_ps = pbc.tile([128, OUT_SH], F32, tag="rps")
                    nc.tensor.matmul(r_ps[:], ones_sb[:], rrow[0:1, ki, :],
                                     start=True, stop=True)
                    # w = w0 + delta  (exact fp32)
                    v = deq.tile([128, OUT_SH], F32, tag="v")
                    nc.vector.tensor_tensor(v[:], d_ps[:], w0_sb[:, ki, :],
                                            AluOpType.add)
                    # v = w * (1/s)
                    nc.vector.tensor_tensor(v[:], v[:], r_ps[:], AluOpType.mult)
                    # clip to [-8, 7]
                    nc.vector.tensor_scalar(v[:], v[:], Q_N, Q_P,
                                            AluOpType.max, AluOpType.min)
                    # round half-to-even
                    nc.vector.tensor_scalar(v[:], v[:], MAGIC, MAGIC,
                                            AluOpType.add, AluOpType.subtract)
                    # w_q = q * s, cast to fp16
                    nc.vector.tensor_tensor(wtq[:, kl, :], v[:], s_ps[:],
                                            AluOpType.mult)

            # ---------- quarter 0 dequant (head) ----------
            wtq_tiles[0] = wtqpool.tile([128, QK, OUT_SH], F16, tag="wtq", name="wtq0")
            for pair in range(QK // 2):
                bufs = deq_dma(0, pair)
                deq_k(0, pair, 0, bufs)
                deq_k(0, pair, 1, bufs)

            # ---------- pipelined GEMM ----------
            for q in range(NQ):
                wtq = wtq_tiles[q]
                for c in range(N_CHUNKS):
                    xs = xpool.tile([128, QK, TOK_CHUNK], F16, tag="xs")
                    nc.sync.dma_start(
                        xs[:],
                        xT_r[:, q * QK:(q + 1) * QK,
                             c * TOK_CHUNK:(c + 1) * TOK_CHUNK])
                    if q + 1 < NQ and c == 0:
                        wtq_tiles[q + 1] = wtqpool.tile(
                            [128, QK, OUT_SH], F16, tag="wtq",
                            name=f"wtq{q + 1}")
                    for t in range(TOK_CHUNK // 128):
                        y_ps = pmm.tile([128, OUT_SH], F32, tag="yps")
                        for j in range(QK):
                            nc.tensor.matmul(y_ps[:],
                                             xs[:, j, t * 128:(t + 1) * 128],
                                             wtq[:, j, :],
                                             start=(j == 0), stop=(j == QK - 1))
                        tt = c * 4 + t
                        if q == 0:
                            # first quarter: fold bias in, write fp16 partials
                            nc.vector.tensor_tensor(y16[:, tt, :], y_ps[:],
                                                    bias_bc[:], AluOpType.add)
                        elif q < NQ - 1:
                            nc.vector.tensor_tensor(y16[:, tt, :], y_ps[:],
                                                    y16[:, tt, :], AluOpType.add)
                        else:
                            yo = ypool.tile([128, OUT_SH], F32, tag="yo")
                            nc.vector.tensor_tensor(yo[:], y_ps[:],
                                                    y16[:, tt, :], AluOpType.add)
                            nc.sync.dma_start(y_r[tt], yo[:])
                        # interleave next quarter's dequant between token
                        # tiles of the first 4 chunks: one k-tile per slot at
                        # t in {0, 2}, DMAs issued a chunk ahead at t == 0
                        if q + 1 < NQ and c < 4 and t == 0:
                            deq_bufs = deq_dma(q + 1, c)
                        if q + 1 < NQ and c < 4 and t in (0, 2):
                            deq_k(q + 1, c, t // 2, deq_bufs)
    nc.compile()
    return nc


def _make_in_maps(x, w0, lora_a, lora_b, q_scale, bias):
    # host-side layout marshalling (no arithmetic beyond the fp16 cast of x,
    # which is the kernel's chosen input precision for the tensor engine)
    x = np.ascontiguousarray(np.asarray(x, dtype=np.float32))
    xT16 = np.ascontiguousarray(x.reshape(M_TOK, IN_F).T).astype(np.float16)
    w0T = np.ascontiguousarray(np.asarray(w0, dtype=np.float32).T)
    lbT = np.ascontiguousarray(np.asarray(lora_b, dtype=np.float32).T)
    qs2 = np.asarray(q_scale, dtype=np.float32).reshape(OUT_F, N_KT)
    bias = np.asarray(bias, dtype=np.float32)
    lora_a = np.ascontiguousarray(np.asarray(lora_a, dtype=np.float32))
    in_maps = []
    for c in range(N_CORES):
        sl = slice(c * OUT_SH, (c + 1) * OUT_SH)
        in_maps.append({
            "xT16": xT16,
            "w0T": np.ascontiguousarray(w0T[:, sl]),
            "lora_a": lora_a,
            "lora_bT": np.ascontiguousarray(lbT[:, sl]),
            "qscT": np.ascontiguousarray(qs2[sl].T),
            "bias": np.ascontiguousarray(bias[sl]).reshape(1, OUT_SH),
        })
    return in_maps


def kernel(x, w0, lora_a, lora_b, q_scale, bias):
    if "nc" not in _CACHE:
        _CACHE["nc"] = _build()
    in_maps = _make_in_maps(x, w0, lora_a, lora_b, q_scale, bias)
    res = run_bass_kernel_spmd(_CACHE["nc"], in_maps,
                               core_ids=list(range(N_CORES)))
    y = np.concatenate([res.results[c]["y"] for c in range(N_CORES)], axis=1)
    return y.reshape(B, S, OUT_F)


def timed_run(inputs):
    """Profiled run for test.py: returns max-core HW exec time in ns."""
    if "nc" not in _CACHE:
        _CACHE["nc"] = _build()
    in_maps = _make_in_maps(**inputs)
    res = run_bass_kernel_spmd(
        _CACHE["nc"], in_maps, core_ids=list(range(N_CORES)),
        trace=True, trace_cores=[0])
    if res.instructions_and_trace:
        insts, path = res.instructions_and_trace
        print("trace path:", path)
        if insts:
            t0 = min(i.timestamp for i in insts)
            t1 = max(i.end_timestamp for i in insts)
            span = t1 - t0
            from collections import defaultdict, Counter
            busy = defaultdict(int)
            cnt = defaultdict(int)
            for i in insts:
                busy[i.engine] += i.duration
                cnt[i.engine] += 1
            print(f"span: {span} ns")
            for e in sorted(busy, key=lambda e: -busy[e]):
                print(f"  {e:>12}: busy {busy[e]:>9} ns ({100.0*busy[e]/span:5.1f}%)"
                      f"  n={cnt[e]}")
            pe = sorted((i for i in insts if i.engine == "TensorMatrix"),
                        key=lambda i: i.timestamp)
            if pe:
                durs = np.array([i.duration for i in pe])
                print("PE dur histogram:",
                      Counter((durs // 50 * 50).tolist()).most_common(10))
                gaps = np.array([b.timestamp - a.end_timestamp
                                 for a, b in zip(pe, pe[1:])])
                gaps = gaps[gaps > 0]
                print(f"PE gaps>0: n={len(gaps)} total={gaps.sum()} "
                      f"max={gaps.max() if len(gaps) else 0}")
                print(f"PE first inst at t+{pe[0].timestamp - t0}, "
                      f"last ends at t+{pe[-1].end_timestamp - t0}")
    return res.exec_time_ns


# revision 10
# speedup vs baseline: 1.2866x; 1.2523x over previous
"""Trainium2 Bass kernel for L4Q quantized linear (LoRA + group fake-quant + GEMM).

Computation (per reference):
    w   = w0 + lora_b @ lora_a                      # [4096, 4096]
    w_q = round(clip(w/s, -8, 7)) * s               # group-wise (groups of 128 along in)
    y   = x @ w_q.T + bias                          # x: [4, 2048, 4096]

Sharding: column-parallel over out_features across 8 cores (512 outs/core).
x is replicated (pre-transposed + fp16-cast on host); each core computes
y[:, :, c*512:(c+1)*512] and the host concatenates.

Numeric strategy:
  - dequant runs in exact fp32 on-device (PE fp32 matmul for the K=16 LoRA
    delta, DVE IEEE fp32 elementwise, magic-number round-half-even,
    NR-refined reciprocal) so quantization decisions match the fp32 reference
    to ~1 ulp.
  - the big GEMM runs in fp16 (11-bit mantissa) with fp32 PSUM accumulation.

Pipelining (v2): K is split into 4 quarters of 8 k-tiles. Quarter 0 is
dequantized up front; quarter q+1's dequant (DVE-heavy) is interleaved under
quarter q's GEMM matmuls so the PE never idles long enough for the HAM clock
throttle to drop it out of the 2.4 GHz p-state (the v1 kernel spent ~150us
running dequant matmuls at the 0.65 GHz cold p-state, fully serialized before
the GEMM). Partial sums accumulate chunk-wise in an fp16 SBUF accumulator.
"""
import numpy as np

import concourse.bass as bass
import concourse.bacc as bacc
import concourse.mybir as mybir
from concourse.tile import TileContext
from concourse.bass_utils import run_bass_kernel_spmd
from concourse.alu_op_type import AluOpType

F32 = mybir.dt.float32
F16 = mybir.dt.float16
MAGIC = 12582912.0  # 1.5 * 2**23: forces round-to-nearest-even at integer granularity

N_CORES = 8
IN_F = 4096
OUT_F = 4096
RANK = 16
B, S = 4, 2048
M_TOK = B * S            # 8192 tokens
OUT_SH = OUT_F // N_CORES  # 512 out features per core
GROUP = 128
N_KT = IN_F // GROUP       # 32 k-tiles (1 quant group per k-tile)
NQ = 4                     # K quarters
QK = N_KT // NQ            # 8 k-tiles per quarter
TOK_CHUNK = 512            # tokens per x-slab DMA
N_CHUNKS = M_TOK // TOK_CHUNK  # 16
N_TT = M_TOK // 128        # 64 token tiles
Q_N, Q_P = -8.0, 7.0

_CACHE = {}


def _build():
    nc = bacc.Bacc(None, target_bir_lowering=False)
    xT_d = nc.dram_tensor("xT16", [IN_F, M_TOK], F16, kind="ExternalInput")
    w0T_d = nc.dram_tensor("w0T", [IN_F, OUT_SH], F32, kind="ExternalInput")
    la_d = nc.dram_tensor("lora_a", [RANK, IN_F], F32, kind="ExternalInput")
    lbT_d = nc.dram_tensor("lora_bT", [RANK, OUT_SH], F32, kind="ExternalInput")
    qsT_d = nc.dram_tensor("qscT", [N_KT, OUT_SH], F32, kind="ExternalInput")
    bias_d = nc.dram_tensor("bias", [1, OUT_SH], F32, kind="ExternalInput")
    y_d = nc.dram_tensor("y", [M_TOK, OUT_SH], F32, kind="ExternalOutput")

    with TileContext(nc) as tc:
        with (
            tc.tile_pool(name="persist", bufs=1) as persist,
            tc.tile_pool(name="wtq", bufs=2) as wtqpool,
            tc.tile_pool(name="w0", bufs=2) as w0pool,
            tc.tile_pool(name="deq", bufs=2) as deq,
            tc.tile_pool(name="xslab", bufs=2) as xpool,
            tc.tile_pool(name="yout", bufs=2) as ypool,
            tc.tile_pool(name="bc", bufs=2) as bc,
            tc.tile_pool(name="pdeq", bufs=2, space="PSUM") as pdeq,
            tc.tile_pool(name="pmm", bufs=4, space="PSUM") as pmm,
            tc.tile_pool(name="dram", bufs=1, space="DRAM") as dram,
        ):
            # ---------- setup ----------
            ones_sb = persist.tile([1, 128], F32)
            nc.vector.memset(ones_sb[:], 1.0)

            la_sb = persist.tile([RANK, IN_F], F32)
            nc.sync.dma_start(la_sb[:], la_d[:, :])
            lbT_sb = persist.tile([RANK, OUT_SH], F32)
            nc.sync.dma_start(lbT_sb[:], lbT_d[:, :])

            # scales: s [32, 512]; r = 1/s via reciprocal + 2 NR (0-ulp exact
            # per HW probe)
            sT32 = persist.tile([N_KT, OUT_SH], F32)
            nc.sync.dma_start(sT32[:], qsT_d[:, :])
            r32 = persist.tile([N_KT, OUT_SH], F32)
            nc.vector.reciprocal(r32[:], sT32[:])
            t32 = persist.tile([N_KT, OUT_SH], F32)
            for _ in range(2):
                nc.vector.tensor_tensor(t32[:], sT32[:], r32[:], AluOpType.mult)
                nc.vector.tensor_scalar(t32[:], t32[:], -1.0, 2.0,
                                        AluOpType.mult, AluOpType.add)
                nc.vector.tensor_tensor(r32[:], r32[:], t32[:], AluOpType.mult)
            r_dram = dram.tile([N_KT, OUT_SH], F32)
            nc.sync.dma_start(r_dram[:], r32[:])

            # bias broadcast tile [128, OUT_SH] fp32 (GpSimd partition bcast)
            biasT_sb = persist.tile([1, OUT_SH], F32)
            nc.sync.dma_start(biasT_sb[:], bias_d[:, :])
            bias_bc = persist.tile([128, OUT_SH], F32)
            nc.gpsimd.partition_broadcast(bias_bc[:], biasT_sb[0:1, :])

            # fp16 partial-sum accumulator, one [128, OUT_SH] slab per token tile
            y16 = persist.tile([128, N_TT, OUT_SH], F16)

            w0T_r = w0T_d.rearrange("(kt p) o -> p kt o", p=128)
            xT_r = xT_d.rearrange("(kt p) m -> p kt m", p=128)
            y_r = y_d.rearrange("(n p) o -> n p o", p=128)

            wtq_tiles = {}

            def deq_dma(q, pair):
                """DMA w0/scale/recip rows for k-tiles (q*QK + 2*pair), +1."""
                k0 = q * QK + 2 * pair
                w0_sb = w0pool.tile([128, 2, OUT_SH], F32, tag="w0")
                nc.sync.dma_start(w0_sb[:], w0T_r[:, k0:k0 + 2, :])
                srow = deq.tile([1, 2, OUT_SH], F32, tag="srow")
                nc.sync.dma_start(srow[:], qsT_d[k0:k0 + 2, :][None])
                rrow = deq.tile([1, 2, OUT_SH], F32, tag="rrow")
                nc.sync.dma_start(rrow[:], r_dram[k0:k0 + 2, :][None])
                return w0_sb, srow, rrow

            def deq_k(q, pair, ki, bufs):
                """Dequantize k-tile q*QK + 2*pair + ki into wtq_tiles[q]."""
                w0_sb, srow, rrow = bufs
                wtq = wtq_tiles[q]
                k = q * QK + 2 * pair + ki
                kl = 2 * pair + ki
                if True:
                    # lora delta^T tile via fp32 PE matmul (K=16)
                    d_ps = pdeq.tile([128, OUT_SH], F32, tag="dps")
                    nc.tensor.matmul(d_ps[:], la_sb[:, k * 128:(k + 1) * 128],
                                     lbT_sb[:], start=True, stop=True)
                    # broadcast scale/recip rows k to 128 partitions (GpSimd)
                    s_bc = bc.tile([128, OUT_SH], F32, tag="sbc")
                    nc.gpsimd.partition_broadcast(s_bc[:], srow[0:1, ki, :])
                    r_bc = bc.tile([128, OUT_SH], F32, tag="rbc")
                    nc.gpsimd.partition_broadcast(r_bc[:], rrow[0:1, ki, :])
                    # w = w0 + delta  (exact fp32)
                    v = deq.tile([128, OUT_SH], F32, tag="v")
                    nc.vector.tensor_tensor(v[:], d_ps[:], w0_sb[:, ki, :],
                                            AluOpType.add)
                    # v = w * (1/s)
                    nc.vector.tensor_tensor(v[:], v[:], r_bc[:], AluOpType.mult)
                    # clip to [-8, 7]
                    nc.vector.tensor_scalar(v[:], v[:], Q_N, Q_P,
                                            AluOpType.max, AluOpType.min)
                    # round half-to-even
                    nc.vector.tensor_scalar(v[:], v[:], MAGIC, MAGIC,
                                            AluOpType.add, AluOpType.subtract)
                    # w_q = q * s, cast to fp16
                    nc.vector.tensor_tensor(wtq[:, kl, :], v[:], s_bc[:],
                                            AluOpType.mult)

            # ---------- quarter 0 dequant (head) ----------
            wtq_tiles[0] = wtqpool.tile([128, QK, OUT_SH], F16, tag="wtq", name="wtq0")
            for pair in range(QK // 2):
                bufs = deq_dma(0, pair)
                deq_k(0, pair, 0, bufs)
                deq_k(0, pair, 1, bufs)

            # ---------- pipelined GEMM ----------
            for q in range(NQ):
                wtq = wtq_tiles[q]
                for c in range(N_CHUNKS):
                    xs = xpool.tile([128, QK, TOK_CHUNK], F16, tag="xs")
                    nc.sync.dma_start(
                        xs[:],
                        xT_r[:, q * QK:(q + 1) * QK,
                             c * TOK_CHUNK:(c + 1) * TOK_CHUNK])
                    if q + 1 < NQ and c == 0:
                        wtq_tiles[q + 1] = wtqpool.tile(
                            [128, QK, OUT_SH], F16, tag="wtq",
                            name=f"wtq{q + 1}")
                    for t in range(TOK_CHUNK // 128):
                        y_ps = pmm.tile([128, OUT_SH], F32, tag="yps")
                        for j in range(QK):
                            nc.tensor.matmul(y_ps[:],
                                             xs[:, j, t * 128:(t + 1) * 128],
                                             wtq[:, j, :],
                                             start=(j == 0), stop=(j == QK - 1))
                        tt = c * 4 + t
                        if q == 0:
                            # first quarter: fold bias in, write fp16 partials
                            nc.vector.tensor_tensor(y16[:, tt, :], y_ps[:],
                                                    bias_bc[:], AluOpType.add)
                        elif q < NQ - 1:
                            nc.vector.tensor_tensor(y16[:, tt, :], y_ps[:],
                                                    y16[:, tt, :], AluOpType.add)
                        else:
                            yo = ypool.tile([128, OUT_SH], F32, tag="yo")
                            nc.vector.tensor_tensor(yo[:], y_ps[:],
                                                    y16[:, tt, :], AluOpType.add)
                            nc.sync.dma_start(y_r[tt], yo[:])
                        # interleave next quarter's dequant between token
                        # tiles of the first 4 chunks: one k-tile per slot at
                        # t in {0, 2}, DMAs issued a chunk ahead at t == 0
                        if q + 1 < NQ and c < 4 and t == 0:
                            deq_bufs = deq_dma(q + 1, c)
                        if q + 1 < NQ and c < 4 and t in (0, 2):
                            deq_k(q + 1, c, t // 2, deq_bufs)
    nc.compile()
    return nc


def _make_in_maps(x, w0, lora_a, lora_b, q_scale, bias):
    # host-side layout marshalling (no arithmetic beyond the fp16 cast of x,
    # which is the kernel's chosen input precision for the tensor engine)
    x = np.ascontiguousarray(np.asarray(x, dtype=np.float32))
    xT16 = np.ascontiguousarray(x.reshape(M_TOK, IN_F).T).astype(np.float16)
    w0T = np.ascontiguousarray(np.asarray(w0, dtype=np.float32).T)
    lbT = np.ascontiguousarray(np.asarray(lora_b, dtype=np.float32).T)
    qs2 = np.asarray(q_scale, dtype=np.float32).reshape(OUT_F, N_KT)
    bias = np.asarray(bias, dtype=np.float32)
    lora_a = np.ascontiguousarray(np.asarray(lora_a, dtype=np.float32))
    in_maps = []
    for c in range(N_CORES):
        sl = slice(c * OUT_SH, (c + 1) * OUT_SH)
        in_maps.append({
            "xT16": xT16,
            "w0T": np.ascontiguousarray(w0T[:, sl]),
            "lora_a": lora_a,
            "lora_bT": np.ascontiguousarray(lbT[:, sl]),
            "qscT": np.ascontiguousarray(qs2[sl].T),
            "bias": np.ascontiguousarray(bias[sl]).reshape(1, OUT_SH),
        })
    return in_maps


def kernel(x, w0, lora_a, lora_b, q_scale, bias):
    if "nc" not in _CACHE:
        _CACHE["nc"] = _build()
    in_maps = _make_in_maps(x, w0, lora_a, lora_b, q_scale, bias)
    res = run_bass_kernel_spmd(_CACHE["nc"], in_maps,
                               core_ids=list(range(N_CORES)))
    y = np.concatenate([res.results[c]["y"] for c in range(N_CORES)], axis=1)
    return y.reshape(B, S, OUT_F)


def timed_run(inputs):
    """Profiled run for test.py: returns max-core HW exec time in ns."""
    if "nc" not in _CACHE:
        _CACHE["nc"] = _build()
    in_maps = _make_in_maps(**inputs)
    res = run_bass_kernel_spmd(
        _CACHE["nc"], in_maps, core_ids=list(range(N_CORES)),
        trace=True, trace_cores=[0])
    if res.instructions_and_trace:
        insts, path = res.instructions_and_trace
        print("trace path:", path)
        if insts:
            t0 = min(i.timestamp for i in insts)
            t1 = max(i.end_timestamp for i in insts)
            span = t1 - t0
            from collections import defaultdict, Counter
            busy = defaultdict(int)
            cnt = defaultdict(int)
            for i in insts:
                busy[i.engine] += i.duration
                cnt[i.engine] += 1
            print(f"span: {span} ns")
            for e in sorted(busy, key=lambda e: -busy[e]):
                print(f"  {e:>12}: busy {busy[e]:>9} ns ({100.0*busy[e]/span:5.1f}%)"
                      f"  n={cnt[e]}")
            pe = sorted((i for i in insts if i.engine == "TensorMatrix"),
                        key=lambda i: i.timestamp)
            if pe:
                durs = np.array([i.duration for i in pe])
                print("PE dur histogram:",
                      Counter((durs // 50 * 50).tolist()).most_common(10))
                gaps = np.array([b.timestamp - a.end_timestamp
                                 for a, b in zip(pe, pe[1:])])
                gaps = gaps[gaps > 0]
                print(f"PE gaps>0: n={len(gaps)} total={gaps.sum()} "
                      f"max={gaps.max() if len(gaps) else 0}")
                print(f"PE first inst at t+{pe[0].timestamp - t0}, "
                      f"last ends at t+{pe[-1].end_timestamp - t0}")
    return res.exec_time_ns
